# revision 1
# baseline (speedup 1.0000x reference)
"""JT-MPN GNN kernel for 8 trn2 NeuronCores (self-contained).

Strategy: edges are dealt round-robin by line-graph in-degree across the 8
cores (data parallel); each BP iteration computes the per-core shard of msg
and AllGathers it (bf16). Segment-sums are done with degree-sorted 128-row
bins + per-round indirect DMA gathers and DVE adds; matmuls run on PE in bf16
with f32 PSUM accumulation. The graph mean-pool is a matmul against a
host-built selection matrix with 1/count weights baked in.
"""
import numpy as np
import ml_dtypes

# Problem constants (hardcoded per contract)
N_NODES = 150000
N_EDGES = 300000
N_LG = 600000
N_TREE = 60000
H = 256
DEPTH = 4
AF = 35
BF = 5
NG = 2048
CORES = 8

EPC = 37504               # edges per core (padded, 293 bins * 128)
NBINS_E = EPC // 128      # 293
GPC = NG // CORES         # 256 graphs per core
SUP = 2                   # supertiles per core (128 graphs each)
ZR = 37500                # global new-edge id of guaranteed-zero msg row (core0 pad slot0)
ZN = N_NODES              # zero row appended to node_x
ZT = N_TREE               # zero row appended to tree_bf


def _group_by(dst, n_groups):
    """Return (order, starts) so that src indices sorted by dst are
    order[starts[g]:starts[g+1]] for group g."""
    order = np.argsort(dst, kind="stable")
    counts = np.bincount(dst, minlength=n_groups)
    starts = np.zeros(n_groups + 1, dtype=np.int64)
    np.cumsum(counts, out=starts[1:])
    return order, starts


def preprocess(inputs):
    edge_src = np.asarray(inputs["edge_src"], dtype=np.int64)
    edge_dst = np.asarray(inputs["edge_dst"], dtype=np.int64)
    lg_src = np.asarray(inputs["lg_src"], dtype=np.int64)
    lg_dst = np.asarray(inputs["lg_dst"], dtype=np.int64)
    tgt_nodes = np.asarray(inputs["tgt_nodes"], dtype=np.int64)
    graph_ids = np.asarray(inputs["graph_ids"], dtype=np.int64)

    meta = {}

    # ---- edge -> core assignment (snake deal by lg in-degree desc) ----
    deg = np.bincount(lg_dst, minlength=N_EDGES)
    order = np.argsort(-deg, kind="stable")      # edges by lg-indegree desc
    pos = np.arange(N_EDGES)
    cyc = pos % (2 * CORES)
    core_of_rank = np.where(cyc < CORES, cyc, 2 * CORES - 1 - cyc)
    # slots[c] = orig edge ids in deg-desc order for core c, padded to EPC with -1
    slots = np.full((CORES, EPC), -1, dtype=np.int64)
    for c in range(CORES):
        mine = order[core_of_rank == c]
        assert len(mine) == N_EDGES // CORES
        slots[c, :len(mine)] = mine
    # new edge id: position of orig edge in its core's slot array
    new_id = np.full(N_EDGES, -1, dtype=np.int64)
    for c in range(CORES):
        valid = slots[c] >= 0
        new_id[slots[c][valid]] = c * EPC + np.nonzero(valid)[0]
    assert (new_id >= 0).all()
    meta["slots"] = slots
    meta["new_id"] = new_id

    # ---- lg rounds: per core, bin, round -> src new-id per slot ----
    lg_order, lg_starts = _group_by(lg_dst, N_EDGES)   # lg edges grouped by dst (orig)
    lg_deg = deg

    # per-core slot degrees
    slot_deg = np.where(slots >= 0, lg_deg[np.clip(slots, 0, None)], 0)
    R_lg = np.zeros(NBINS_E, dtype=np.int64)
    for b in range(NBINS_E):
        R_lg[b] = slot_deg[:, b*128:(b+1)*128].max()
    meta["R_lg"] = R_lg.tolist()
    CLG = int(R_lg.sum())
    lgidx = np.full((CORES, 128, CLG), ZR, dtype=np.int32)
    col = 0
    for b in range(NBINS_E):
        for r in range(int(R_lg[b])):
            for c in range(CORES):
                sl = slots[c, b*128:(b+1)*128]
                for d in range(128):
                    e = sl[d]
                    if e >= 0 and r < lg_deg[e]:
                        j = lg_order[lg_starts[e] + r]
                        lgidx[c, d, col] = new_id[lg_src[j]]
            col += 1
    assert col == CLG
    meta["CLG"] = CLG

    # ---- tree rounds for edges (alphaE = sum of tree rows targeting src(e)) ----
    t_order, t_starts = _group_by(tgt_nodes, N_NODES)  # tree rows grouped by target node
    t_cnt = np.bincount(tgt_nodes, minlength=N_NODES)
    # per-slot tree count = t_cnt[src(e)]
    esrc_of_slot = np.where(slots >= 0, edge_src[np.clip(slots, 0, None)], ZN)
    slot_tc = np.where(slots >= 0, t_cnt[np.clip(esrc_of_slot, 0, N_NODES - 1)], 0)
    slot_tc = np.where(esrc_of_slot == ZN, 0, slot_tc)
    R_te = np.zeros(NBINS_E, dtype=np.int64)
    for b in range(NBINS_E):
        R_te[b] = slot_tc[:, b*128:(b+1)*128].max()
    meta["R_te"] = R_te.tolist()
    CTE = int(R_te.sum())
    teidx = np.full((CORES, 128, max(CTE, 1)), ZT, dtype=np.int32)
    col = 0
    for b in range(NBINS_E):
        for r in range(int(R_te[b])):
            for c in range(CORES):
                sl = slots[c, b*128:(b+1)*128]
                for d in range(128):
                    e = sl[d]
                    if e >= 0:
                        v = edge_src[e]
                        if r < t_cnt[v]:
                            teidx[c, d, col] = t_order[t_starts[v] + r]
            col += 1
    meta["CTE"] = CTE

    # ---- node_x gather idx for features (per edge slot, src orig id; pad -> ZN) ----
    nxidx = esrc_of_slot.astype(np.int32)  # [CORES, EPC], ZN for pads
    nxidx = nxidx.reshape(CORES, NBINS_E, 128).transpose(0, 2, 1)  # [CORES,128,NBINS_E]

    # ---- node -> (core, supertile, bin, slot) ----
    g_starts = np.zeros(NG + 1, dtype=np.int64)
    np.cumsum(np.bincount(graph_ids, minlength=NG), out=g_starts[1:])
    n_deg = np.bincount(edge_dst, minlength=N_NODES)
    counts_g = np.bincount(graph_ids, minlength=NG).astype(np.float64)

    sup_nodes = []   # list of arrays of orig node ids per (core, sup)
    for c in range(CORES):
        for u in range(SUP):
            g0 = c * GPC + u * 128
            nodes = np.arange(g_starts[g0], g_starts[g0 + 128])
            # sort by in-degree desc for homogeneous bins
            nodes = nodes[np.argsort(-n_deg[nodes], kind="stable")]
            sup_nodes.append(nodes)
    NBINS_N = int(max((len(x) + 127) // 128 for x in sup_nodes))
    meta["NBINS_N"] = NBINS_N
    NPS = NBINS_N * 128          # node slots per supertile
    nslot = np.full((CORES, SUP, NPS), -1, dtype=np.int64)
    for c in range(CORES):
        for u in range(SUP):
            nodes = sup_nodes[c * SUP + u]
            nslot[c, u, :len(nodes)] = nodes

    # m rounds (incoming edges by new id) + treeN rounds per node bin
    e_order, e_starts = _group_by(edge_dst, N_NODES)
    slot_nd = np.where(nslot >= 0, n_deg[np.clip(nslot, 0, None)], 0)
    slot_nt = np.where(nslot >= 0, t_cnt[np.clip(nslot, 0, None)], 0)
    R_m = np.zeros((SUP * NBINS_N,), dtype=np.int64)   # shared over cores, indexed u*NBINS_N+b
    R_tn = np.zeros((SUP * NBINS_N,), dtype=np.int64)
    for u in range(SUP):
        for b in range(NBINS_N):
            R_m[u * NBINS_N + b] = slot_nd[:, u, b*128:(b+1)*128].max()
            R_tn[u * NBINS_N + b] = slot_nt[:, u, b*128:(b+1)*128].max()
    meta["R_m"] = R_m.tolist()
    meta["R_tn"] = R_tn.tolist()
    CM = int(R_m.sum())
    CTN = int(R_tn.sum())
    midx = np.full((CORES, 128, max(CM, 1)), ZR, dtype=np.int32)
    tnidx = np.full((CORES, 128, max(CTN, 1)), ZT, dtype=np.int32)
    colm = 0
    coltn = 0
    for u in range(SUP):
        for b in range(NBINS_N):
            for r in range(int(R_m[u * NBINS_N + b])):
                for c in range(CORES):
                    sl = nslot[c, u, b*128:(b+1)*128]
                    for d in range(128):
                        v = sl[d]
                        if v >= 0 and r < n_deg[v]:
                            e = e_order[e_starts[v] + r]
                            midx[c, d, colm] = new_id[e]
                colm += 1
            for r in range(int(R_tn[u * NBINS_N + b])):
                for c in range(CORES):
                    sl = nslot[c, u, b*128:(b+1)*128]
                    for d in range(128):
                        v = sl[d]
                        if v >= 0 and r < t_cnt[v]:
                            tnidx[c, d, coltn] = t_order[t_starts[v] + r]
                coltn += 1
    meta["CM"] = CM
    meta["CTN"] = CTN

    # ---- per-core data tensors ----
    node_x = np.asarray(inputs["node_x"], dtype=np.float32)
    bond_x = np.asarray(inputs["bond_x"], dtype=np.float32)
    node_x_dev = np.concatenate([node_x, np.zeros((1, AF), np.float32)], axis=0)

    bond_xT = np.zeros((CORES, BF, EPC), dtype=np.float32)
    for c in range(CORES):
        valid = slots[c] >= 0
        bond_xT[c, :, valid] = bond_x[slots[c][valid]].astype(np.float32)

    # node_xT per core [36, SUP*NPS] (+ ones row for bias)
    node_xT = np.zeros((CORES, AF + 1, SUP * NPS), dtype=np.float32)
    spool = np.zeros((CORES, SUP * NPS, 128), dtype=np.float32)
    inv_cnt = 1.0 / np.maximum(counts_g, 1.0)
    for c in range(CORES):
        for u in range(SUP):
            sl = nslot[c, u]
            valid = sl >= 0
            base = u * NPS
            node_xT[c, :AF, base:base + NPS][:, valid] = node_x[sl[valid]].T
            node_xT[c, AF, base:base + NPS][valid] = 1.0
            gl = graph_ids[np.clip(sl, 0, None)] - (c * GPC + u * 128)  # local graph 0..127
            for d in range(NPS):
                if sl[d] >= 0:
                    spool[c, base + d, gl[d]] = inv_cnt[graph_ids[sl[d]]]

    W_i = np.asarray(inputs["W_i"], dtype=np.float32)
    W_h = np.asarray(inputs["W_h"], dtype=np.float32)
    W_o = np.asarray(inputs["W_o"], dtype=np.float32)
    b_o = np.asarray(inputs["b_o"], dtype=np.float32)
    Wo_top_ext = np.concatenate([W_o[:AF], b_o[None, :]], axis=0)  # [36, 256]
    tree_pad = np.zeros((60416, H), dtype=np.float32)
    tree_pad[:N_TREE] = np.asarray(inputs["tree_mess"], dtype=np.float32)

    per_core = []
    for c in range(CORES):
        per_core.append({
            "node_x_dev": node_x_dev,
            "tree_mess": tree_pad,
            "bond_xT": bond_xT[c],
            "node_xT": node_xT[c],
            "spool": spool[c],
            "Wi": W_i,
            "Wh_bf": W_h.astype(ml_dtypes.bfloat16),
            "Wo_top_ext": Wo_top_ext,
            "Wo_bot_bf": W_o[AF:].astype(ml_dtypes.bfloat16),
            "ident_f32": np.eye(128, dtype=np.float32),
            "ident_bf": np.eye(128).astype(ml_dtypes.bfloat16),
            "lgidx": np.ascontiguousarray(lgidx[c]),
            "teidx": np.ascontiguousarray(teidx[c]),
            "nxidx": np.ascontiguousarray(nxidx[c]),
            "midx": np.ascontiguousarray(midx[c]),
            "tnidx": np.ascontiguousarray(tnidx[c]),
        })
    meta["NPS"] = NPS
    return per_core, meta


def emulate(per_core, meta, bf16=True):
    """Numpy emulation of the kernel dataflow; returns [NG, H] float32."""
    def q(x):
        return x.astype(ml_dtypes.bfloat16).astype(np.float32) if bf16 else x

    R_lg, R_te = meta["R_lg"], meta["R_te"]
    R_m, R_tn = meta["R_m"], meta["R_tn"]
    NBINS_N, NPS = meta["NBINS_N"], meta["NPS"]

    E_ALL = CORES * EPC
    out = np.zeros((NG, H), np.float32)

    # device-wide msg buffer (simulating AG'd full msg)
    msg = np.zeros((E_ALL, H), np.float32)
    input2_all = []
    tree_bf_all = []
    for c in range(CORES):
        pc = per_core[c]
        tree_bf = np.concatenate([q(pc["tree_mess"]), np.zeros((1, H), np.float32)], axis=0)
        tree_bf_all.append(tree_bf)
        Wh = pc["Wh_bf"].astype(np.float32)
        # phase 1+2
        feats = np.concatenate([
            pc["node_x_dev"][pc["nxidx"].transpose(1, 0).reshape(-1)],   # [EPC, 35] slot-major
            pc["bond_xT"].T], axis=1)                                    # [EPC, 40]
        msg_input = feats @ pc["Wi"]                                     # [EPC, 256] f32
        msg0 = q(np.maximum(msg_input, 0.0))
        # alphaE
        alphaE = np.zeros((EPC, H), np.float32)
        col = 0
        for b in range(NBINS_E):
            for r in range(R_te[b]):
                rows = tree_bf[pc["teidx"][:, col]]          # [128, H] bf16 values
                a = alphaE[b*128:(b+1)*128]
                alphaE[b*128:(b+1)*128] = q(a + rows)
                col += 1
        input2 = q(msg_input + q(alphaE) @ Wh)
        input2_all.append(input2)
        msg[c*EPC:(c+1)*EPC] = msg0

    for it in range(DEPTH - 1):
        new_msg = np.zeros_like(msg)
        for c in range(CORES):
            pc = per_core[c]
            Wh = pc["Wh_bf"].astype(np.float32)
            col = 0
            accum = np.zeros((EPC, H), np.float32)
            for b in range(NBINS_E):
                a = np.zeros((128, H), np.float32)
                for r in range(R_lg[b]):
                    rows = msg[pc["lgidx"][:, col]]
                    a = q(a + rows) if r > 0 else rows.copy()
                    col += 1
                accum[b*128:(b+1)*128] = a
            delta = q(accum) @ Wh
            new_msg[c*EPC:(c+1)*EPC] = q(np.maximum(input2_all[c] + delta, 0.0))
        msg = new_msg

    for c in range(CORES):
        pc = per_core[c]
        Wo_bot = pc["Wo_bot_bf"].astype(np.float32)
        colm = 0
        coltn = 0
        for u in range(SUP):
            g_psum = np.zeros((128, H), np.float32)
            for b in range(NBINS_N):
                t2 = np.zeros((128, H), np.float32)
                for r in range(R_m[u * NBINS_N + b]):
                    rows = msg[pc["midx"][:, colm]]
                    t2 = q(t2 + rows) if r > 0 else rows.copy()
                    colm += 1
                for r in range(R_tn[u * NBINS_N + b]):
                    rows = tree_bf_all[c][pc["tnidx"][:, coltn]]
                    t2 = q(t2 + rows)
                    coltn += 1
                base = u * NPS + b * 128
                nxT = pc["node_xT"][:, base:base + 128]       # [36, 128]
                ps = nxT.T @ pc["Wo_top_ext"] + q(t2) @ Wo_bot
                h = np.maximum(ps, 0.0)                        # [128, 256] f32
                sp = pc["spool"][base:base + 128]              # [128, 128]
                g_psum += sp.T @ h
            out[c * GPC + u * 128: c * GPC + (u + 1) * 128] = g_psum
    return out


import numpy as np
import concourse.bacc as bacc
import concourse.bass as bass
import concourse.mybir as mybir
import concourse.tile as tile
from concourse.bass import IndirectOffsetOnAxis

F32 = mybir.dt.float32
BF16 = mybir.dt.bfloat16
I32 = mybir.dt.int32
AluOp = mybir.AluOpType
Act = mybir.ActivationFunctionType

# constants mirrored from prep
EPC = 37504
NBINS_E = 293
CORES = 8
H = 256
AF = 35
SUP = 2
GPC = 256
TREE_PAD = 60416
N_NODES = 150000
DEPTH = 4


def build(meta, n_iters=DEPTH - 1, n_groups_limit=None, stub_collectives=False):
    """Build the full SPMD kernel. meta comes from prep.preprocess."""
    R_lg = meta["R_lg"]
    R_te = meta["R_te"]
    R_m = meta["R_m"]
    R_tn = meta["R_tn"]
    NBINS_N = meta["NBINS_N"]
    NPS = NBINS_N * 128
    CLG = max(meta["CLG"], 1)
    CTE = max(meta["CTE"], 1)
    CM = max(meta["CM"], 1)
    CTN = max(meta["CTN"], 1)

    nc = bacc.Bacc("TRN2", target_bir_lowering=False, debug=False)

    # ---- external IO ----
    node_x_dev = nc.dram_tensor("node_x_dev", [N_NODES + 1, AF], F32, kind="ExternalInput")
    tree_mess = nc.dram_tensor("tree_mess", [TREE_PAD, H], F32, kind="ExternalInput")
    bond_xT = nc.dram_tensor("bond_xT", [5, EPC], F32, kind="ExternalInput")
    node_xT = nc.dram_tensor("node_xT", [AF + 1, SUP * NPS], F32, kind="ExternalInput")
    spool = nc.dram_tensor("spool", [SUP * NPS, 128], F32, kind="ExternalInput")
    Wi = nc.dram_tensor("Wi", [AF + 5, H], F32, kind="ExternalInput")
    Wh_bf = nc.dram_tensor("Wh_bf", [H, H], BF16, kind="ExternalInput")
    Wo_top_ext = nc.dram_tensor("Wo_top_ext", [AF + 1, H], F32, kind="ExternalInput")
    Wo_bot_bf = nc.dram_tensor("Wo_bot_bf", [H, H], BF16, kind="ExternalInput")
    ident_f32 = nc.dram_tensor("ident_f32", [128, 128], F32, kind="ExternalInput")
    ident_bf = nc.dram_tensor("ident_bf", [128, 128], BF16, kind="ExternalInput")
    lgidx = nc.dram_tensor("lgidx", [128, CLG], I32, kind="ExternalInput")
    teidx = nc.dram_tensor("teidx", [128, CTE], I32, kind="ExternalInput")
    nxidx = nc.dram_tensor("nxidx", [128, NBINS_E], I32, kind="ExternalInput")
    midx = nc.dram_tensor("midx", [128, CM], I32, kind="ExternalInput")
    tnidx = nc.dram_tensor("tnidx", [128, CTN], I32, kind="ExternalInput")
    gout = nc.dram_tensor("gout", [GPC, H], F32, kind="ExternalOutput")

    # group layout over edge bins
    groups = []
    b0 = 0
    while b0 < NBINS_E:
        groups.append(list(range(b0, min(b0 + 8, NBINS_E))))
        b0 += 8
    if n_groups_limit is not None:
        groups = groups[:n_groups_limit]

    with tile.TileContext(nc) as tc:
        with tc.tile_pool(name="dram", bufs=1, space="DRAM") as dram, \
             tc.tile_pool(name="const", bufs=1) as cpool, \
             tc.tile_pool(name="psum_t", bufs=2, space="PSUM") as ppt, \
             tc.tile_pool(name="psum_b", bufs=3, space="PSUM") as ppb, \
             tc.tile_pool(name="psum_g", bufs=2, space="PSUM") as ppg, \
             tc.tile_pool(name="sbuf", bufs=3) as sb:

            tree_bf = dram.tile([TREE_PAD, H], BF16)
            msg_shard = dram.tile([EPC, H], BF16)
            n_ags = 1 + n_iters
            msg_fulls = [dram.tile([CORES * EPC, H], BF16,
                                   addr_space=("Local" if stub_collectives else "Shared"),
                                   name=f"msg_full_{k}")
                         for k in range(n_ags)]
            input2_d = dram.tile([EPC, H], BF16)

            # ---- constants ----
            wi_sb = cpool.tile([AF + 5, H], F32)
            nc.sync.dma_start(wi_sb[:], Wi[:])
            whA = cpool.tile([128, H], BF16)
            whB = cpool.tile([128, H], BF16)
            nc.sync.dma_start(whA[:], Wh_bf[0:128, :])
            nc.sync.dma_start(whB[:], Wh_bf[128:256, :])
            wotop = cpool.tile([AF + 1, H], F32)
            nc.sync.dma_start(wotop[:], Wo_top_ext[:])
            wobA = cpool.tile([128, H], BF16)
            wobB = cpool.tile([128, H], BF16)
            nc.sync.dma_start(wobA[:], Wo_bot_bf[0:128, :])
            nc.sync.dma_start(wobB[:], Wo_bot_bf[128:256, :])
            idf = cpool.tile([128, 128], F32)
            idb = cpool.tile([128, 128], BF16)
            nc.sync.dma_start(idf[:], ident_f32[:])
            nc.sync.dma_start(idb[:], ident_bf[:])
            lgidx_sb = cpool.tile([128, CLG], I32)
            nc.sync.dma_start(lgidx_sb[:], lgidx[:])
            teidx_sb = cpool.tile([128, CTE], I32)
            nc.sync.dma_start(teidx_sb[:], teidx[:])
            nxidx_sb = cpool.tile([128, NBINS_E], I32)
            nc.sync.dma_start(nxidx_sb[:], nxidx[:])
            midx_sb = cpool.tile([128, CM], I32)
            nc.sync.dma_start(midx_sb[:], midx[:])
            tnidx_sb = cpool.tile([128, CTN], I32)
            nc.sync.dma_start(tnidx_sb[:], tnidx[:])

            # ---- phase 0.5: cast tree_mess f32 -> bf16 ----
            for ch in range(TREE_PAD // 1024):
                rows = tree_mess[ch * 1024:(ch + 1) * 1024, :]
                tin = sb.tile([128, 8, H], F32, tag="tcast_in")
                nc.sync.dma_start(tin[:], rows.rearrange("(m p) d -> p m d", p=128))
                tout = sb.tile([128, 8, H], BF16, tag="tcast_out")
                nc.vector.tensor_copy(tout[:].rearrange("p m d -> p (m d)"),
                                      tin[:].rearrange("p m d -> p (m d)"))
                nc.sync.dma_start(
                    tree_bf[ch * 1024:(ch + 1) * 1024, :].rearrange("(m p) d -> p m d", p=128),
                    tout[:])

            # ---- helper: gather rounds + add ----
            def gather_rounds(idx_sb, src_dram, cols, dt, tag):
                """Gather len(cols) rounds of [128, H] rows; returns accum tile."""
                acc = sb.tile([128, H], dt, tag=f"{tag}_acc")
                for i, col in enumerate(cols):
                    if i == 0:
                        nc.gpsimd.indirect_dma_start(
                            out=acc[:], out_offset=None, in_=src_dram[:],
                            in_offset=IndirectOffsetOnAxis(ap=idx_sb[:, col:col + 1], axis=0))
                    else:
                        g = sb.tile([128, H], dt, tag=f"{tag}_g")
                        nc.gpsimd.indirect_dma_start(
                            out=g[:], out_offset=None, in_=src_dram[:],
                            in_offset=IndirectOffsetOnAxis(ap=idx_sb[:, col:col + 1], axis=0))
                        nc.vector.tensor_tensor(out=acc[:], in0=acc[:], in1=g[:], op=AluOp.add)
                return acc

            def transpose_to(acc, ident, tag):
                """acc [128, 256] -> TT bf16 [128, 256] with halves transposed."""
                TT = sb.tile([128, H], BF16, tag=f"{tag}_tt")
                for half in range(2):
                    pt = ppt.tile([128, 128], acc.dtype, tag="pt")
                    nc.tensor.transpose(pt[:], acc[:, half * 128:(half + 1) * 128], ident[:])
                    nc.scalar.activation(TT[:, half * 128:(half + 1) * 128], pt[:], Act.Copy)
                return TT

            # ---- phase 1+2 fused ----
            col_te = 0
            for grp in groups:
                gsz = len(grp)
                g0 = grp[0]
                featTb = sb.tile([AF + 5, 8 * 128], F32, tag="featTb")
                nc.sync.dma_start(featTb[AF:AF + 5, :gsz * 128],
                                  bond_xT[:, g0 * 128:(g0 + gsz) * 128])
                msgb = sb.tile([128, 8, H], BF16, tag="msgb")
                i2b = sb.tile([128, 8, H], BF16, tag="i2b")
                for bi, b in enumerate(grp):
                    gx = sb.tile([128, AF], F32, tag="gx")
                    nc.gpsimd.indirect_dma_start(
                        out=gx[:], out_offset=None, in_=node_x_dev[:],
                        in_offset=IndirectOffsetOnAxis(ap=nxidx_sb[:, b:b + 1], axis=0))
                    pt = ppt.tile([128, 128], F32, tag="pt")
                    nc.tensor.transpose(pt[:AF, :], gx[:], idf[:])
                    nc.scalar.activation(featTb[:AF, bi * 128:(bi + 1) * 128], pt[:AF, :], Act.Copy)
                    pa = ppb.tile([128, H], F32, tag="ps")
                    nc.tensor.matmul(pa[:], lhsT=featTb[:AF + 5, bi * 128:(bi + 1) * 128],
                                     rhs=wi_sb[:], start=True, stop=True)
                    nc.scalar.activation(msgb[:, bi, :], pa[:], Act.Relu)
                    R = R_te[b]
                    if R > 0:
                        cpa = sb.tile([128, H], F32, tag="cpa")
                        nc.scalar.activation(cpa[:], pa[:], Act.Copy)
                        acc = gather_rounds(teidx_sb, tree_bf,
                                            list(range(col_te, col_te + R)), BF16, "te")
                        col_te += R
                        TT = transpose_to(acc, idb, "te")
                        pb = ppb.tile([128, H], F32, tag="ps")
                        nc.tensor.matmul(pb[:], lhsT=TT[:, 0:128], rhs=whA[:], start=True, stop=False)
                        nc.tensor.matmul(pb[:], lhsT=TT[:, 128:256], rhs=whB[:], start=False, stop=False)
                        nc.tensor.matmul(pb[:], lhsT=idf[:], rhs=cpa[:], start=False, stop=True)
                        nc.scalar.activation(i2b[:, bi, :], pb[:], Act.Copy)
                    else:
                        nc.scalar.activation(i2b[:, bi, :], pa[:], Act.Copy)
                rows = slice(g0 * 128, (g0 + gsz) * 128)
                nc.sync.dma_start(
                    msg_shard[rows, :].rearrange("(m p) d -> p m d", p=128), msgb[:, :gsz, :])
                nc.sync.dma_start(
                    input2_d[rows, :].rearrange("(m p) d -> p m d", p=128), i2b[:, :gsz, :])

            def allgather(k):
                if stub_collectives:
                    # timing proxy: ~2 shard-sized DMA copies approximate the
                    # measured 8-core AllGather cost (~90us for 154MB)
                    for rep in range(2):
                        nc.sync.dma_start(
                            msg_fulls[k][rep * EPC:(rep + 1) * EPC, :], msg_shard[:])
                    return
                nc.gpsimd.collective_compute(
                    "AllGather", AluOp.bypass,
                    replica_groups=[list(range(CORES))],
                    ins=[msg_shard[:].opt()],
                    outs=[msg_fulls[k][:].opt()])

            allgather(0)

            # ---- BP iterations ----
            for it in range(n_iters):
                col_lg = 0
                for grp in groups:
                    gsz = len(grp)
                    g0 = grp[0]
                    rows = slice(g0 * 128, (g0 + gsz) * 128)
                    i2l = sb.tile([128, 8, H], BF16, tag="i2l")
                    nc.sync.dma_start(i2l[:, :gsz, :],
                                      input2_d[rows, :].rearrange("(m p) d -> p m d", p=128))
                    msgb = sb.tile([128, 8, H], BF16, tag="msgb2")
                    for bi, b in enumerate(grp):
                        R = R_lg[b]
                        if R > 0:
                            acc = gather_rounds(lgidx_sb, msg_fulls[it],
                                                list(range(col_lg, col_lg + R)), BF16, "lg")
                            col_lg += R
                            TT = transpose_to(acc, idb, "lg")
                            pb = ppb.tile([128, H], F32, tag="ps")
                            nc.tensor.matmul(pb[:], lhsT=TT[:, 0:128], rhs=whA[:], start=True, stop=False)
                            nc.tensor.matmul(pb[:], lhsT=TT[:, 128:256], rhs=whB[:], start=False, stop=False)
                            nc.tensor.matmul(pb[:], lhsT=idb[:], rhs=i2l[:, bi, :], start=False, stop=True)
                            nc.scalar.activation(msgb[:, bi, :], pb[:], Act.Relu)
                        else:
                            nc.scalar.activation(msgb[:, bi, :], i2l[:, bi, :], Act.Relu)
                    nc.sync.dma_start(
                        msg_shard[rows, :].rearrange("(m p) d -> p m d", p=128),
                        msgb[:, :gsz, :])
                allgather(it + 1)

            # ---- final: nodes, W_o, pooling ----
            colm = 0
            coltn = 0
            for u in range(SUP):
                pg = ppg.tile([128, H], F32, tag="pg")
                for b in range(NBINS_N):
                    Rm = R_m[u * NBINS_N + b]
                    Rtn = R_tn[u * NBINS_N + b]
                    cols_m = list(range(colm, colm + Rm))
                    cols_tn = list(range(coltn, coltn + Rtn))
                    colm += Rm
                    coltn += Rtn
                    if Rm > 0:
                        t2 = gather_rounds(midx_sb, msg_fulls[n_iters], cols_m, BF16, "m")
                        for i, col in enumerate(cols_tn):
                            g = sb.tile([128, H], BF16, tag="tn_g")
                            nc.gpsimd.indirect_dma_start(
                                out=g[:], out_offset=None, in_=tree_bf[:],
                                in_offset=IndirectOffsetOnAxis(ap=tnidx_sb[:, col:col + 1], axis=0))
                            nc.vector.tensor_tensor(out=t2[:], in0=t2[:], in1=g[:], op=AluOp.add)
                    elif Rtn > 0:
                        t2 = gather_rounds(tnidx_sb, tree_bf, cols_tn, BF16, "m")
                    else:
                        t2 = sb.tile([128, H], BF16, tag="m_acc")
                        nc.vector.memset(t2[:], 0.0)
                    base = u * NPS + b * 128
                    nxg = sb.tile([AF + 1, 128], F32, tag="nxg")
                    nc.sync.dma_start(nxg[:], node_xT[:, base:base + 128])
                    pc_ = ppb.tile([128, H], F32, tag="ps")
                    nc.tensor.matmul(pc_[:], lhsT=nxg[:], rhs=wotop[:], start=True, stop=False)
                    TT = transpose_to(t2, idb, "m")
                    nc.tensor.matmul(pc_[:], lhsT=TT[:, 0:128], rhs=wobA[:], start=False, stop=False)
                    nc.tensor.matmul(pc_[:], lhsT=TT[:, 128:256], rhs=wobB[:], start=False, stop=True)
                    h = sb.tile([128, H], F32, tag="h")
                    nc.scalar.activation(h[:], pc_[:], Act.Relu)
                    sp = sb.tile([128, 128], F32, tag="sp")
                    nc.sync.dma_start(sp[:], spool[base:base + 128, :])
                    nc.tensor.matmul(pg[:], lhsT=sp[:], rhs=h[:],
                                     start=(b == 0), stop=(b == NBINS_N - 1))
                go = sb.tile([128, H], F32, tag="go")
                nc.scalar.activation(go[:], pg[:], Act.Copy)
                nc.sync.dma_start(gout[u * 128:(u + 1) * 128, :], go[:])

    nc.finalize()
    return nc


def make_in_maps(per_core):
    names = ["node_x_dev", "tree_mess", "bond_xT", "node_xT", "spool", "Wi", "Wh_bf",
             "Wo_top_ext", "Wo_bot_bf", "ident_f32", "ident_bf",
             "lgidx", "teidx", "nxidx", "midx", "tnidx"]
    return [{n: np.ascontiguousarray(pc[n]) for n in names} for pc in per_core]


_BUILD_CACHE = {}


def kernel(**inputs):
    import numpy as _np
    from concourse import bass_utils as _bass_utils

    per_core, meta = preprocess(inputs)
    key = (meta["CLG"], meta["CTE"], meta["CM"], meta["CTN"], meta["NBINS_N"],
           tuple(meta["R_lg"]), tuple(meta["R_te"]))
    nc = _BUILD_CACHE.get(key)
    if nc is None:
        nc = build(meta)
        _BUILD_CACHE[key] = nc
    in_maps = make_in_maps(per_core)
    res = _bass_utils.run_bass_kernel_spmd(nc, in_maps, core_ids=list(range(CORES)))
    out = _np.concatenate([res.results[c]["gout"] for c in range(CORES)], axis=0)
    return out.astype(_np.float32)

